# revision 1
# baseline (speedup 1.0000x reference)
"""Trainium2 Bass kernel for nn_DenoiseNet (retrieval_knn).

Per-core work (data-parallel over batch B=8 across 8 NeuronCores):
one batch's full denoising loss:
  for module i in 0..3:
    target_i = centered_clean + noise_i * std/4^(i+1)   (i<2), else centered_clean
    s[n,m]/2 = q_n.t_m - ||t_m||^2/2   (argmax_m s == argmin_m ||q_n - t_m||^2)
    m*(n)    = argmax_m s[n,m]                          (DVE max8 + max_index)
    nb       = t[m*]                                    (indirect DMA gather)
    q       += disp_i
    dist_n   = ||q_n - nb_n||^2
    loss_i   = sum_n dist_n
Host sums the 8 per-core [4] losses, divides by B, returns (loss, loss).

The s rows are computed on the Vector engine as a 3-op scalar_tensor_tensor
chain against partition-broadcast target-coordinate rows (t0/t1/t2 and
-||t||^2/2, replicated to all 128 partitions via a DRAM-bounce broadcast DMA),
with the per-query coordinates fed as per-partition scalars. This beats the
PE-matmul formulation on this system because execution cost here is dominated
by a ~25-50us per-instruction overhead: 6 instructions per 128-query tile
(3 stt + max8 + max_index + gather) instead of ~30 (18 matmuls + 9 PSUM->SBUF
copies + ...). Measured ~60ms per-core execution vs ~140ms for the matmul
variant; loss matches the jax reference to ~3e-7 relative.
"""

import os
import sys

import numpy as np

for _p in ("/opt/trn_rl_repo",):
    if os.path.isdir(_p) and _p not in sys.path:
        sys.path.insert(0, _p)

import bass_rust
import concourse.bass as bass
import concourse.mybir as mybir
from concourse.bass_utils import run_bass_kernel_spmd
from concourse.tile import TileContext

F32 = mybir.dt.float32
AX = mybir.AxisListType
OP = mybir.AluOpType

B, N, M, D = 8, 4096, 4608, 3
NT = N // 128           # 32 n-tiles
MC = M // 512           # 9 m-chunks
NMOD = 4

# ---------------------------------------------------------------------------
# Workaround: this container's walrus build supports only ONE sync-wait
# command per instruction. Split every multi-wait instruction by inserting
# same-engine NoOps (each carrying one wait) immediately before it.
# ---------------------------------------------------------------------------


def _split_multi_waits(nc):
    counter = 0
    for f in nc.m.functions:
        for blk in f.blocks:
            il = blk.instructions
            i = 0
            while i < len(il):
                inst = il[i]
                si = inst.sync_info
                if si is not None and si.on_wait and len(si.on_wait) > 1:
                    waits = list(si.on_wait)
                    for w in waits[:-1]:
                        counter += 1
                        nop = mybir.InstNoOp(
                            name=f"Wsplit-{counter}",
                            ins=[],
                            outs=[],
                            engine=inst.engine,
                        )
                        nop.sync_info = bass_rust.SyncInfo(on_wait=[w], on_update=[])
                        il.insert(i, nop)
                        i += 1
                    si.on_wait = [waits[-1]]
                i += 1
    return counter


# ---------------------------------------------------------------------------
# Kernel build
# ---------------------------------------------------------------------------


def _build():
    nc = bass.Bass()

    # Per-core inputs (pure layout transforms of one batch's tensors).
    noisyT = nc.dram_tensor("noisyT", [3, N], F32, kind="ExternalInput")
    noisy_nat = nc.dram_tensor("noisy_nat", [128, NT * 3], F32, kind="ExternalInput")
    cleanT = nc.dram_tensor("cleanT", [3, M], F32, kind="ExternalInput")
    clean_nat = nc.dram_tensor("clean_nat", [128, (M // 128) * 3], F32, kind="ExternalInput")
    seedT = nc.dram_tensor("seedT", [3, 1], F32, kind="ExternalInput")
    seed_nat96 = nc.dram_tensor("seed_nat96", [128, NT * 3], F32, kind="ExternalInput")
    seed_nat108 = nc.dram_tensor("seed_nat108", [128, (M // 128) * 3], F32, kind="ExternalInput")
    std3 = nc.dram_tensor("std3", [3, 1], F32, kind="ExternalInput")
    std_nat = nc.dram_tensor("std_nat", [128, 1], F32, kind="ExternalInput")
    dispT = nc.dram_tensor("dispT", [3, NMOD * N], F32, kind="ExternalInput")
    disp_nat = nc.dram_tensor("disp_nat", [128, NMOD * NT * 3], F32, kind="ExternalInput")
    noiseT = nc.dram_tensor("noiseT", [6, M], F32, kind="ExternalInput")
    noise_nat = nc.dram_tensor("noise_nat", [128, 2 * (M // 128) * 3], F32, kind="ExternalInput")

    loss_out = nc.dram_tensor("loss4", [4, 1], F32, kind="ExternalOutput")

    # Gather tables (row-major [M, 3]) — indirect DMA requires offset-0 tensors.
    tgt_tables = [
        nc.dram_tensor(f"tgt_table{i}", [M, 3], F32, kind="Internal")
        for i in range(3)  # module 0, module 1, modules 2&3 (clean)
    ]

    MCH = M // 128  # 36 chunks of 128 along m for the nat layout

    rows_dram = [
        nc.dram_tensor(f"rows_dram{i}", [3, M], F32, kind="Internal")
        for i in range(3)
    ]

    with TileContext(nc) as tc:
        with (
            tc.tile_pool(name="cst", bufs=1) as cst,
            tc.tile_pool(name="sbig", bufs=2) as sbig,
            tc.tile_pool(name="ps_small", bufs=1, space="PSUM") as psp_small,
            tc.tile_pool(name="work", bufs=4) as work,
        ):
            # ---------------- static tiles -----------------
            t_seedT = cst.tile([3, 1], F32)
            t_std3 = cst.tile([3, 1], F32)
            t_seed108 = cst.tile([128, MCH * 3], F32)
            t_dispnat = cst.tile([128, NMOD * NT * 3], F32)
            t_noisenat = cst.tile([128, 2 * MCH * 3], F32)
            t_cleannat = cst.tile([128, MCH * 3], F32)
            t_stdnat = cst.tile([128, 1], F32)
            t_seed96 = cst.tile([128, NT * 3], F32)

            t_sig = cst.tile([3, 2], F32)
            t_signat = cst.tile([128, 2], F32)
            t_losscols = cst.tile([128, 4], F32)
            t_ones128 = cst.tile([128, 1], F32)
            t_cleanTc = cst.tile([3, M], F32)    # centered clean (transposed)
            t_rows = cst.tile([3, M], F32)       # target rows staging
            # broadcast target rows + (-n2/2) row
            t_b = [cst.tile([128, M], F32, name=f"brow{d}") for d in range(3)]
            t_n2b = cst.tile([128, M], F32)
            # query in nat layout: two alternating buffers (old/new)
            t_q = [cst.tile([128, NT * 3], F32, name=f"qnat{j}") for j in range(2)]

            for dst, srcp in (
                (t_seedT, seedT), (t_std3, std3),
                (t_seed96, seed_nat96), (t_seed108, seed_nat108),
                (t_dispnat, disp_nat), (t_noisenat, noise_nat),
                (t_cleannat, clean_nat), (t_stdnat, std_nat),
                (t_q[0], noisy_nat), (t_cleanTc, cleanT),
            ):
                nc.sync.dma_start(dst[:], srcp[:])

            nc.vector.memset(t_ones128[:], 1.0)

            # sigma columns: std/4, std/16 (exact powers of two)
            nc.vector.tensor_scalar(t_sig[:, 0:1], t_std3[:], 0.25, None, OP.mult)
            nc.vector.tensor_scalar(t_sig[:, 1:2], t_sig[:, 0:1], 0.25, None, OP.mult)
            nc.vector.tensor_scalar(t_signat[:, 0:1], t_stdnat[:], 0.25, None, OP.mult)
            nc.vector.tensor_scalar(t_signat[:, 1:2], t_signat[:, 0:1], 0.25, None, OP.mult)

            # centered query (nat) and centered clean (both layouts)
            nc.vector.tensor_tensor(out=t_q[0][:], in0=t_q[0][:], in1=t_seed96[:],
                                    op=OP.subtract)
            nc.vector.tensor_scalar(t_cleanTc[:], t_cleanTc[:], t_seedT[:], None,
                                    OP.subtract)
            nc.vector.tensor_tensor(out=t_cleannat[:], in0=t_cleannat[:],
                                    in1=t_seed108[:], op=OP.subtract)

            # ---------------- gather tables (nat layout -> DRAM) -----------
            t_tgtnat = [cst.tile([128, MCH * 3], F32, name=f"tgtnat{i}") for i in range(2)]
            for i in range(2):
                nc.vector.tensor_scalar(t_tgtnat[i][:],
                                        t_noisenat[:, i * MCH * 3:(i + 1) * MCH * 3],
                                        t_signat[:, i:i + 1], None, OP.mult)
                nc.gpsimd.tensor_tensor(out=t_tgtnat[i][:], in0=t_tgtnat[i][:],
                                        in1=t_cleannat[:], op=OP.add)
            for i in range(3):
                srct = t_tgtnat[i] if i < 2 else t_cleannat
                dview = tgt_tables[i][:].rearrange("(c p) d -> p c d", p=128)
                sview = srct[:].rearrange("p (c d) -> p c d", d=3)
                nc.sync.dma_start(dview, sview)

            # ---------------- per-module loop ----------------
            qold, qnew = t_q[0], t_q[1]
            for i in range(NMOD):
                tgt_tab = tgt_tables[min(i, 2)]

                if i < 2:
                    # target rows (transposed): noise*sigma + centered clean
                    nc.sync.dma_start(t_rows[:], noiseT[3 * i:3 * i + 3, :])
                    nc.vector.tensor_scalar(t_rows[:], t_rows[:],
                                            t_sig[:, i:i + 1], None, OP.mult)
                    nc.gpsimd.tensor_tensor(out=t_rows[:], in0=t_rows[:],
                                            in1=t_cleanTc[:], op=OP.add)
                    nc.sync.dma_start(rows_dram[i][:], t_rows[:])
                elif i == 2:
                    nc.sync.dma_start(rows_dram[2][:], t_cleanTc[:])

                if i != 3:
                    # broadcast rows to all 128 partitions via DRAM bounce,
                    # then build -0.5*||t||^2 with 3 stt + 2 adds
                    rd = rows_dram[min(i, 2)]
                    for d in range(3):
                        nc.sync.dma_start(t_b[d][:],
                                          rd[d:d + 1, :].to_broadcast([128, M]))
                    t_tmp = work.tile([128, M], F32, tag="n2tmp", bufs=1)
                    nc.vector.scalar_tensor_tensor(
                        out=t_n2b[:], in0=t_b[0][:], scalar=-0.5,
                        in1=t_b[0][:], op0=OP.mult, op1=OP.mult)
                    nc.vector.scalar_tensor_tensor(
                        out=t_tmp[:], in0=t_b[1][:], scalar=-0.5,
                        in1=t_b[1][:], op0=OP.mult, op1=OP.mult)
                    nc.vector.tensor_tensor(out=t_n2b[:], in0=t_n2b[:],
                                            in1=t_tmp[:], op=OP.add)
                    nc.vector.scalar_tensor_tensor(
                        out=t_tmp[:], in0=t_b[2][:], scalar=-0.5,
                        in1=t_b[2][:], op0=OP.mult, op1=OP.mult)
                    nc.vector.tensor_tensor(out=t_n2b[:], in0=t_n2b[:],
                                            in1=t_tmp[:], op=OP.add)

                # qnew = qold + disp_i (dist uses post-update query)
                nc.vector.tensor_tensor(
                    out=qnew[:], in0=qold[:],
                    in1=t_dispnat[:, i * NT * 3:(i + 1) * NT * 3], op=OP.add)

                t_nball = work.tile([128, NT * 3], F32, tag="nball", bufs=2)
                for t in range(NT):
                    # s/2 = q.t - ||t||^2/2 as a 3-op stt chain per tile
                    t_x1 = sbig.tile([128, M], F32, tag="xrow1")
                    t_x2 = sbig.tile([128, M], F32, tag="xrow2", bufs=1)
                    nc.vector.scalar_tensor_tensor(
                        out=t_x2[:], in0=t_b[0][:], scalar=qold[:, 3 * t:3 * t + 1],
                        in1=t_n2b[:], op0=OP.mult, op1=OP.add)
                    nc.vector.scalar_tensor_tensor(
                        out=t_x1[:], in0=t_b[1][:], scalar=qold[:, 3 * t + 1:3 * t + 2],
                        in1=t_x2[:], op0=OP.mult, op1=OP.add)
                    nc.vector.scalar_tensor_tensor(
                        out=t_x2[:], in0=t_b[2][:], scalar=qold[:, 3 * t + 2:3 * t + 3],
                        in1=t_x1[:], op0=OP.mult, op1=OP.add)

                    t_max8 = work.tile([128, 8], F32, tag="max8")
                    t_idx8 = work.tile([128, 8], mybir.dt.uint32, tag="idx8")
                    nc.vector.max(t_max8[:], t_x2[:])
                    nc.vector.max_index(t_idx8[:], t_max8[:], t_x2[:])

                    nc.gpsimd.indirect_dma_start(
                        out=t_nball[:, t * 3:(t + 1) * 3], out_offset=None,
                        in_=tgt_tab[:],
                        in_offset=bass.IndirectOffsetOnAxis(ap=t_idx8[:, 0:1], axis=0),
                    )

                # dist = ||qnew - nb||^2, batched over the whole module
                t_diffall = work.tile([128, NT * 3], F32, tag="diffall", bufs=2)
                t_dall = work.tile([128, NT], F32, tag="dall")
                nc.vector.tensor_tensor(out=t_diffall[:], in0=qnew[:],
                                        in1=t_nball[:], op=OP.subtract)
                nc.scalar.activation(t_diffall[:], t_diffall[:],
                                     mybir.ActivationFunctionType.Square)
                nc.vector.tensor_reduce(
                    out=t_dall[:], in_=t_diffall[:].rearrange("p (t d) -> p t d", d=3),
                    axis=AX.X, op=OP.add)
                nc.vector.tensor_reduce(out=t_losscols[:, i:i + 1], in_=t_dall[:],
                                        axis=AX.X, op=OP.add)

                qold, qnew = qnew, qold

            # sum over partitions: [4,1] = loss_cols.T @ ones
            ps_loss = psp_small.tile([4, 1], F32, tag="pssmall")
            nc.tensor.matmul(ps_loss[:], t_losscols[:], t_ones128[:],
                             start=True, stop=True)
            t_loss = work.tile([4, 1], F32, tag="lossout")
            nc.scalar.copy(t_loss[:], ps_loss[:])
            nc.sync.dma_start(loss_out[:], t_loss[:])

    _split_multi_waits(nc)
    return nc


_NC_CACHE = None


def _get_nc():
    global _NC_CACHE
    if _NC_CACHE is None:
        _NC_CACHE = _build()
    return _NC_CACHE


# ---------------------------------------------------------------------------
# Host-side sharding (pure layout) and gather of per-core results
# ---------------------------------------------------------------------------


def _shard(b, pcl_noisy, pcl_clean, pcl_seeds, pcl_std, pred_disp, noise):
    f32 = np.float32
    noisy = np.ascontiguousarray(pcl_noisy[b], dtype=f32)        # (N,3)
    clean = np.ascontiguousarray(pcl_clean[b], dtype=f32)        # (M,3)
    seed = np.ascontiguousarray(pcl_seeds[b, 0], dtype=f32)      # (3,)
    disp = np.ascontiguousarray(pred_disp[:, b], dtype=f32)      # (4,N,3)
    noi = np.ascontiguousarray(noise[:, b], dtype=f32)           # (2,M,3)
    MCH = M // 128
    return {
        "noisyT": np.ascontiguousarray(noisy.T),
        "noisy_nat": np.ascontiguousarray(
            noisy.reshape(NT, 128, 3).transpose(1, 0, 2).reshape(128, NT * 3)),
        "cleanT": np.ascontiguousarray(clean.T),
        "clean_nat": np.ascontiguousarray(
            clean.reshape(MCH, 128, 3).transpose(1, 0, 2).reshape(128, MCH * 3)),
        "seedT": np.ascontiguousarray(seed.reshape(3, 1)),
        "seed_nat96": np.ascontiguousarray(np.tile(seed, (128, NT))),
        "seed_nat108": np.ascontiguousarray(np.tile(seed, (128, MCH))),
        "std3": np.full((3, 1), pcl_std[b], dtype=f32),
        "std_nat": np.full((128, 1), pcl_std[b], dtype=f32),
        "dispT": np.ascontiguousarray(
            disp.transpose(2, 0, 1).reshape(3, NMOD * N)),
        "disp_nat": np.ascontiguousarray(
            disp.reshape(NMOD, NT, 128, 3).transpose(2, 0, 1, 3).reshape(128, NMOD * NT * 3)),
        "noiseT": np.ascontiguousarray(noi.transpose(0, 2, 1).reshape(6, M)),
        "noise_nat": np.ascontiguousarray(
            noi.reshape(2, MCH, 128, 3).transpose(2, 0, 1, 3).reshape(128, 2 * MCH * 3)),
    }


_LAST_EXEC_NS = None


def kernel(pcl_noisy, pcl_clean, pcl_seeds, pcl_std, pred_disp, noise,
           trace=False):
    global _LAST_EXEC_NS
    nc = _get_nc()
    in_maps = [
        _shard(b, pcl_noisy, pcl_clean, pcl_seeds, pcl_std, pred_disp, noise)
        for b in range(B)
    ]
    res = run_bass_kernel_spmd(nc, in_maps, core_ids=list(range(B)), trace=trace)
    _LAST_EXEC_NS = res.exec_time_ns
    total = np.float64(0.0)
    per_mod = np.zeros(4, dtype=np.float64)
    for b in range(B):
        per_mod += res.results[b]["loss4"][:, 0].astype(np.float64)
    loss = np.float32((per_mod / B).sum())
    return (loss, loss)



# revision 49
# speedup vs baseline: 5.2604x; 5.2604x over previous
"""Trainium2 Bass kernel for nn_DenoiseNet (retrieval_knn).

Data-parallel over batch B=8 across 8 NeuronCores; each core computes one
batch's full 4-module denoising loss.

Per module i (target set j = min(i,2)):
  s[n,m] = q_n . t_m - ||t_m||^2 / 2          (argmax_m s == argmin_m ||q-t||^2)
  m*(n)  = argmax_m s[n,m]
  q      += disp_i
  loss_i = sum_n ||q_n - t_{m*(n)}||^2

Implementation (per 128-query tile, M=4608 targets):
  - PE computes s via f32r matmuls (K=4: q0,q1,q2,ones x t0,t1,t2,-.5||t||^2)
    into PSUM pieces A(2048)/B(2048)/C(512); C reuses A's banks.
  - pass 1 (hierarchical group-max over 72 blocks of 64 columns):
    Pool does a tensor_tensor max tree on A (64->8 per block), DVE finishes A
    and flat-reduces B and C; DVE max8+max_index picks the winning block g*.
  - one batched indirect DMA per 16 tiles gathers block rows
    {x,y,z,-.5||t||^2} of each query's winning block.
  - pass 2 (within-block, exact fp32): recompute the 64 scores vs q_old
    (Pool stt chain), DVE max8 -> winner value; mask = (s==max);
    scores vs q_new, masked dot-accumulate gives s_new[k*]; then
    dist_n = ||q_new||^2 - 2 s_new[k*], accumulated per module.
  - loss4 = per-module partition sums via a final PE matmul with ones.

Host side only shards/relayouts inputs and sums the per-core loss4 outputs.
"""

import os
import sys

import numpy as np

for _p in ("/opt/trn_rl_repo",):
    if os.path.isdir(_p) and _p not in sys.path:
        sys.path.insert(0, _p)

import bass_rust
import concourse.bass as bass
import concourse.mybir as mybir
from concourse.bass_utils import run_bass_kernel_spmd
from concourse.tile import TileContext

F32 = mybir.dt.float32
F32R = mybir.dt.float32r
F16 = mybir.dt.float16
U32 = mybir.dt.uint32
AX = mybir.AxisListType
OP = mybir.AluOpType

B, N, M, D = 8, 4096, 4608, 3
NT = N // 128            # 32 query tiles
NMOD = 4
HALF = 16                # tiles per gather batch
K = 64                   # block size (columns per group)
G = M // K               # 72 blocks
MA, MB, MC = 2048, 2048, 512   # piece sizes: A (pool tree), B (DVE), C
GA, GB, GC = MA // K, MB // K, MC // K   # 32, 32, 8 blocks

# Tunable build configuration (see sweep.py): structural/emission knobs.
CFG = {
    "p2_pool_chains": 1,     # 0/1/2 of the two pass-2 chains decomposed on Pool
    "late_prep": True,       # defer sets 1-2 target prep past module 0 emission
    "ma": 1536,              # A piece size
    "lag": 8,                # pass-2 emission lag behind pass 1, in tiles
    "qt_on_pool": True,      # qT/qnat updates on Pool (tensor_tensor add)
    "tree_levels": 4,        # DVE fp16 tree levels on drained pieces
}


# ---------------------------------------------------------------------------
# Workaround: this container's walrus build supports only ONE sync-wait
# command per instruction. Split every multi-wait instruction by inserting
# same-engine NoOps (each carrying one wait) immediately before it.
# ---------------------------------------------------------------------------


def _split_multi_waits(nc):
    counter = 0
    for f in nc.m.functions:
        for blk in f.blocks:
            il = blk.instructions
            i = 0
            while i < len(il):
                inst = il[i]
                si = inst.sync_info
                if si is not None and si.on_wait and len(si.on_wait) > 1:
                    waits = list(si.on_wait)
                    for w in waits[:-1]:
                        counter += 1
                        nop = mybir.InstNoOp(
                            name=f"Wsplit-{counter}",
                            ins=[],
                            outs=[],
                            engine=inst.engine,
                        )
                        nop.sync_info = bass_rust.SyncInfo(on_wait=[w], on_update=[])
                        il.insert(i, nop)
                        i += 1
                    si.on_wait = [waits[-1]]
                i += 1
    return counter


# ---------------------------------------------------------------------------
# Kernel build
# ---------------------------------------------------------------------------


def _build(cfg=None):
    global MA, MB, GA, GB
    if cfg:
        CFG.update(cfg)
    MA = CFG.get("ma", 2048)
    MB = M - MA - MC
    GA, GB = MA // K, MB // K
    nc = bass.Bass()

    qT0 = nc.dram_tensor("qT0", [4, N], F32R, kind="ExternalInput")
    cleanT = nc.dram_tensor("cleanT", [3, M], F32R, kind="ExternalInput")
    seedT = nc.dram_tensor("seedT", [3, 1], F32, kind="ExternalInput")
    std3 = nc.dram_tensor("std3", [3, 1], F32, kind="ExternalInput")
    noiseT = nc.dram_tensor("noiseT", [6, M], F32, kind="ExternalInput")
    neghalf3 = nc.dram_tensor("neghalf3", [3, 1], F32R, kind="ExternalInput")
    dispT = nc.dram_tensor("dispT", [3, NMOD * N], F32, kind="ExternalInput")
    noisy_nat = nc.dram_tensor("noisy_nat", [128, NT * 3], F32, kind="ExternalInput")
    seed_nat96 = nc.dram_tensor("seed_nat96", [128, NT * 3], F32, kind="ExternalInput")
    disp_nat = nc.dram_tensor("disp_nat", [128, NMOD * NT * 3], F32,
                              kind="ExternalInput")

    loss_out = nc.dram_tensor("loss4", [4, 1], F32, kind="ExternalOutput")

    blocks_dram = [
        nc.dram_tensor(f"blocks_dram{j}", [G, 4 * K], F32, kind="Internal")
        for j in range(3)
    ]

    with TileContext(nc) as tc:
        with (
            tc.tile_pool(name="cst", bufs=1) as cst,
            tc.tile_pool(name="ps", bufs=1, space="PSUM") as psp,
            tc.tile_pool(name="work", bufs=2) as work,
        ):
            # ---------------- static tiles -----------------
            t_qT = cst.tile([4, N], F32R)
            t_rows = [cst.tile([4, M], F32R, name=f"rows{j}") for j in range(3)]
            t_seedT = cst.tile([3, 1], F32)
            t_sig = cst.tile([3, 2], F32)
            t_std3 = cst.tile([3, 1], F32)
            t_neghalf = cst.tile([3, 1], F32R)
            t_ones = cst.tile([128, 1], F32)
            t_seed96 = cst.tile([128, NT * 3], F32)
            t_dispnat = cst.tile([128, NMOD * NT * 3], F32)
            t_qnat = [cst.tile([128, NT * 3], F32, name=f"qnat{k}")
                      for k in range(NMOD + 1)]
            t_losspart = cst.tile([128, 4], F32)

            nc.sync.dma_start(t_qT[:], qT0[:])
            nc.sync.dma_start(t_rows[2][0:3, :], cleanT[:])
            nc.sync.dma_start(t_seedT[:], seedT[:])
            nc.sync.dma_start(t_std3[:], std3[:])
            nc.sync.dma_start(t_seed96[:], seed_nat96[:])
            nc.sync.dma_start(t_dispnat[:], disp_nat[:])
            nc.sync.dma_start(t_qnat[0][:], noisy_nat[:])
            nc.sync.dma_start(t_neghalf[:], neghalf3[:])

            nc.vector.memset(t_ones[:], 1.0)

            # sigma columns: std/4, std/16
            nc.vector.tensor_scalar(t_sig[:, 0:1], t_std3[:], 0.25, None, OP.mult)
            nc.vector.tensor_scalar(t_sig[:, 1:2], t_sig[:, 0:1], 0.25, None,
                                    OP.mult)

            # center queries and clean targets on the seed
            nc.vector.tensor_scalar(t_qT[0:3, :], t_qT[0:3, :].bitcast(F32),
                                    t_seedT[:], None, OP.subtract)
            nc.vector.tensor_scalar(t_rows[2][0:3, :],
                                    t_rows[2][0:3, :].bitcast(F32),
                                    t_seedT[:], None, OP.subtract)
            nc.vector.tensor_tensor(out=t_qnat[0][:], in0=t_qnat[0][:],
                                    in1=t_seed96[:], op=OP.subtract)

            # ---------------- rows + n2 + block tables (prologue) ----------
            scr_pool = tc.tile_pool(name="scr", bufs=1)
            scr = scr_pool.__enter__()

            def emit_noise_rows(j):
                t_noise = scr.tile([4, M], F32, tag="noise", bufs=1,
                                   name=f"noise{j}")
                nc.sync.dma_start(t_noise[0:3, :], noiseT[3 * j:3 * j + 3, :])
                nc.vector.scalar_tensor_tensor(
                    out=t_rows[j][0:3, :], in0=t_noise[0:3, :],
                    scalar=t_sig[:, j:j + 1],
                    in1=t_rows[2][0:3, :].bitcast(F32),
                    op0=OP.mult, op1=OP.add)

            def emit_set_prep(j):
                t_sq = scr.tile([4, M], F32R, tag="sq", bufs=1, name=f"sq{j}")
                t_n2s = t_sq
                nc.scalar.copy(t_sq[0:3, :],
                               t_rows[j][0:3, :].bitcast(F32))
                nc.scalar.square(t_sq[0:3, :],
                                 t_sq[0:3, :].bitcast(F32))

                pw = min(2048, MB)
                for piece in range(M // pw + (1 if M % pw else 0)):
                    lo = piece * pw
                    hi = min(M, lo + pw)
                    pn2 = psp.tile([128, MB], F32, tag="pB",
                                   bufs=1, name=f"pn2_{j}")
                    for c in range(lo, hi, 512):
                        nc.tensor.matmul(
                            pn2[0:1, c - lo:c - lo + 512],
                            t_neghalf[:],
                            t_sq[0:3, c:c + 512],
                            start=True, stop=True)
                    nc.scalar.copy(t_n2s[0:1, lo:hi], pn2[0:1, 0:hi - lo])
                nc.sync.dma_start(t_rows[j][3:4, :], t_n2s[0:1, :])
                bview = blocks_dram[j][:].rearrange("b (r k) -> r b k", r=4)
                nc.sync.dma_start(
                    bview,
                    t_rows[j][:].bitcast(F32).rearrange("r (b k) -> r b k", k=K))

            emit_noise_rows(0)
            emit_noise_rows(1)
            if CFG.get("late_prep", False):
                emit_set_prep(0)
            else:
                for j in range(3):
                    emit_set_prep(j)
                scr_pool.__exit__(None, None, None)
                scr_pool = None

            # ---------------- main loop ----------------
            pending = None  # deferred pass-2 emission: (i, h, blocks tile, val32)
            val32_of = {}

            def chain_pool(xg, yg, zg, n2g, q3, out_t, nm):
                u1 = work.tile([128, K], F32, tag=f"{nm}u1", name=f"{nm}u1")
                u2 = work.tile([128, K], F32, tag=f"{nm}u2", name=f"{nm}u2")
                u3 = work.tile([128, K], F32, tag=f"{nm}u3", name=f"{nm}u3")
                nc.gpsimd.tensor_scalar(u1[:], xg, q3[0], None, OP.mult)
                nc.gpsimd.tensor_scalar(u2[:], yg, q3[1], None, OP.mult)
                nc.gpsimd.tensor_scalar(u3[:], zg, q3[2], None, OP.mult)
                v1 = work.tile([128, K], F32, tag=f"{nm}v1", name=f"{nm}v1")
                nc.gpsimd.tensor_tensor(out=v1[:], in0=u1[:], in1=u2[:], op=OP.add)
                v2 = work.tile([128, K], F32, tag=f"{nm}v2", name=f"{nm}v2")
                nc.gpsimd.tensor_tensor(out=v2[:], in0=u3[:], in1=n2g, op=OP.add)
                nc.gpsimd.tensor_tensor(out=out_t[:], in0=v1[:], in1=v2[:],
                                        op=OP.add)

            def chain_dve(xg, yg, zg, n2g, q3, out_t, nm):
                c1 = work.tile([128, K], F32, tag=f"{nm}c1", name=f"{nm}c1")
                c2 = work.tile([128, K], F32, tag=f"{nm}c2", name=f"{nm}c2")
                nc.vector.scalar_tensor_tensor(out=c1[:], in0=xg, scalar=q3[0],
                                               in1=n2g, op0=OP.mult, op1=OP.add)
                nc.vector.scalar_tensor_tensor(out=c2[:], in0=yg, scalar=q3[1],
                                               in1=c1[:], op0=OP.mult, op1=OP.add)
                nc.vector.scalar_tensor_tensor(out=out_t[:], in0=zg, scalar=q3[2],
                                               in1=c2[:], op0=OP.mult, op1=OP.add)

            def emit_pass2_tile(i, h, tt, t_blocks, t_val32):
                qold, qnew = t_qnat[i], t_qnat[i + 1]
                t = h * HALF + tt
                xg = t_blocks[:, tt, 0:K]
                yg = t_blocks[:, tt, K:2 * K]
                zg = t_blocks[:, tt, 2 * K:3 * K]
                n2g = t_blocks[:, tt, 3 * K:4 * K]
                qo = [qold[:, 3 * t + d:3 * t + d + 1] for d in range(3)]
                qn = [qnew[:, 3 * t + d:3 * t + d + 1] for d in range(3)]
                npc = CFG["p2_pool_chains"]
                sOld = work.tile([128, K], F32, tag="p2so")
                sNew = work.tile([128, K], F32, tag="p2sn")
                (chain_pool if npc >= 1 else chain_dve)(
                    xg, yg, zg, n2g, qo, sOld, "po")
                so8 = work.tile([128, 8], F32, tag="p2m8")
                nc.vector.max(so8[:], sOld[:])
                (chain_pool if npc >= 2 else chain_dve)(
                    xg, yg, zg, n2g, qn, sNew, "pn")
                trash = work.tile([128, K], F32, tag="p2tr")
                nc.vector.scalar_tensor_tensor(
                    out=trash[:], in0=sOld[:], scalar=so8[:, 0:1], in1=sNew[:],
                    op0=OP.is_equal, op1=OP.mult,
                    accum_out=t_val32[:, t:t + 1])

            def emit_module_tail(i, t_val32):
                # loss_i partials: sum_t (||qnew||^2 - 2 * sNew[k*])
                qnew = t_qnat[i + 1]
                sqn = work.tile([128, NT * 3], F32, tag="sqn")
                nc.scalar.square(sqn[:], qnew[:])
                q2 = work.tile([128, NT], F32, tag="q2")
                nc.vector.tensor_reduce(
                    out=q2[:], in_=sqn[:].rearrange("p (t d) -> p t d", d=3),
                    axis=AX.X, op=OP.add)
                tmp = work.tile([128, NT], F32, tag="lsum")
                nc.vector.scalar_tensor_tensor(
                    out=tmp[:], in0=t_val32[:], scalar=-2.0, in1=q2[:],
                    op0=OP.mult, op1=OP.add)
                nc.vector.tensor_reduce(out=t_losspart[:, i:i + 1], in_=tmp[:],
                                        axis=AX.X, op=OP.add)

            from collections import deque
            p2q = deque()
            units_left = {}

            def pump(limit):
                while len(p2q) > limit:
                    pi, ph, ptt, pb = p2q.popleft()
                    emit_pass2_tile(pi, ph, ptt, pb, val32_of[pi])
                    units_left[pi] -= 1
                    if units_left[pi] == 0:
                        emit_module_tail(pi, val32_of[pi])

            qeng = nc.gpsimd if CFG.get("qt_on_pool", True) else nc.vector

            half_state = {}

            def start_tile(it):
                i, h, tt = it["i"], it["h"], it["tt"]
                rows = t_rows[min(i, 2)]
                t = h * HALF + tt
                lhsT = t_qT[:, 128 * t:128 * (t + 1)]

                def mm(dst, lo, hi):
                    for c in range(lo, hi, 512):
                        nc.tensor.matmul(dst[:, c - lo:c - lo + 512], lhsT,
                                         rows[:, c:c + 512],
                                         start=True, stop=True)

                # ACT drains A and B into one fp16 buffer (frees PSUM fast;
                # DVE runs a single merged max-tree at 2x over both)
                sAB = work.tile([128, MA + MB], F16, tag="sAB")
                pAf = psp.tile([128, MA], F32, tag="pA", name="pA_main")
                mm(pAf, 0, MA)
                nc.scalar.copy(sAB[:, 0:MA], pAf[:])
                pB = psp.tile([128, MB], F32, tag="pB", name="pB_main")
                mm(pB, MA, MA + MB)
                nc.scalar.copy(sAB[:, MA:MA + MB], pB[:])
                pC = psp.tile([128, MA], F32, tag="pA", name="pC_m")
                mm(pC, MA + MB, M)
                it["sAB"], it["pC"] = sAB, pC

            def finish_tile(it):
                i, h, tt = it["i"], it["h"], it["tt"]
                sAB, pC = it["sAB"], it["pC"]
                t_gmax = work.tile([128, G], F32, tag="gmax")
                nc.vector.tensor_reduce(
                    out=t_gmax[:, GA + GB:G],
                    in_=pC[:, 0:MC].rearrange("p (g k) -> p g k", k=K),
                    axis=AX.X, op=OP.max)
                cur, ksz, width = sAB, K, MA + MB
                for lv in range(CFG["tree_levels"]):
                    nxt = work.tile([128, width // 2], F16, tag=f"tL{lv}",
                                    name=f"tL{lv}")
                    vv = cur[:].rearrange("p (g k) -> p g k", k=ksz)
                    nc.vector.tensor_tensor(
                        out=nxt[:].rearrange("p (g k) -> p g k", k=ksz // 2),
                        in0=vv[:, :, 0:ksz // 2], in1=vv[:, :, ksz // 2:ksz],
                        op=OP.max)
                    cur, ksz, width = nxt, ksz // 2, width // 2
                nc.vector.tensor_reduce(
                    out=t_gmax[:, 0:GA + GB],
                    in_=cur[:].rearrange("p (g k) -> p g k", k=ksz),
                    axis=AX.X, op=OP.max)
                m8 = work.tile([128, 8], F32, tag="m8")
                i8 = work.tile([128, 8], U32, tag="i8", bufs=4)
                nc.vector.max(m8[:], t_gmax[:])
                nc.vector.max_index(i8[:], m8[:], t_gmax[:])
                # per-tile single-offset gather ([128,1] offsets: the only
                # indirect-DMA shape that matches HW SWDGE semantics)
                hs = half_state[(i, h)]
                nc.gpsimd.indirect_dma_start(
                    out=hs["blocks"][:, tt, :], out_offset=None,
                    in_=blocks_dram[min(i, 2)][:],
                    in_offset=bass.IndirectOffsetOnAxis(ap=i8[:, 0:1], axis=0),
                )
                p2q.append((i, h, tt, hs["blocks"]))

            items = []
            for i in range(NMOD):
                for h in range(2):
                    for tt in range(HALF):
                        items.append({"i": i, "h": h, "tt": tt})

            prev = None
            for it in items:
                i, h, tt = it["i"], it["h"], it["tt"]
                if tt == 0 and h == 0:
                    # module entry: qnat update + per-module buffers
                    units_left[i] = 2 * HALF
                    val32_of[i] = work.tile([128, NT], F32, tag="val32",
                                            name=f"val32_{i}")
                    qeng.tensor_tensor(
                        out=t_qnat[i + 1][:], in0=t_qnat[i][:],
                        in1=t_dispnat[:, i * NT * 3:(i + 1) * NT * 3],
                        op=OP.add)
                if tt == 0:
                    half_state[(i, h)] = {
                        "blocks": work.tile([128, HALF, 4 * K], F32,
                                            tag="blocks",
                                            name=f"blocks_{i}_{h}"),
                    }
                if prev is not None:
                    finish_tile(prev)
                    pp = prev
                    if pp["tt"] == HALF - 1 and pp["h"] == 1:
                        # module boundary housekeeping after last finish
                        pi = pp["i"]
                        if pi == 0 and CFG.get("late_prep", False):
                            emit_set_prep(1)
                            emit_set_prep(2)
                        if pi + 1 < NMOD:
                            t_dT = work.tile([3, N], F32, tag="dispT", bufs=1)
                            nc.sync.dma_start(t_dT[:],
                                              dispT[:, pi * N:(pi + 1) * N])
                            nc.vector.tensor_tensor(
                                out=t_qT[0:3, :],
                                in0=t_qT[0:3, :].bitcast(F32),
                                in1=t_dT[:], op=OP.add)
                start_tile(it)
                pump(CFG["lag"])
                prev = it
            finish_tile(prev)

            # drain pending pass 2
            pump(0)
            if scr_pool is not None:
                scr_pool.__exit__(None, None, None)
                scr_pool = None

            # final loss: [4,1] = losspart.T @ ones
            ps_loss = psp.tile([128, MB], F32, tag="pB", bufs=1, name="ps_loss")
            nc.tensor.matmul(ps_loss[0:4, 0:1], t_losspart[:], t_ones[:],
                             start=True, stop=True)
            t_loss = work.tile([4, 1], F32, tag="lossout")
            nc.scalar.copy(t_loss[:], ps_loss[0:4, 0:1])
            nc.sync.dma_start(loss_out[:], t_loss[:])

    return nc


_NC_CACHE = None
_NC_SPLIT = False


def _get_nc(split_waits=False, cfg=None):
    global _NC_CACHE, _NC_SPLIT
    if _NC_CACHE is None:
        _NC_CACHE = _build(cfg)
    if split_waits and not _NC_SPLIT:
        _split_multi_waits(_NC_CACHE)
        _NC_SPLIT = True
    return _NC_CACHE


# ---------------------------------------------------------------------------
# Host-side sharding (pure layout) and gather of per-core results
# ---------------------------------------------------------------------------


def _shard(b, pcl_noisy, pcl_clean, pcl_seeds, pcl_std, pred_disp, noise):
    f32 = np.float32
    noisy = np.ascontiguousarray(pcl_noisy[b], dtype=f32)        # (N,3)
    clean = np.ascontiguousarray(pcl_clean[b], dtype=f32)        # (M,3)
    seed = np.ascontiguousarray(pcl_seeds[b, 0], dtype=f32)      # (3,)
    disp = np.ascontiguousarray(pred_disp[:, b], dtype=f32)      # (4,N,3)
    noi = np.ascontiguousarray(noise[:, b], dtype=f32)           # (2,M,3)
    return {
        "qT0": np.ascontiguousarray(
            np.concatenate([noisy.T, np.ones((1, N), dtype=f32)], axis=0)),
        "cleanT": np.ascontiguousarray(clean.T),
        "seedT": np.ascontiguousarray(seed.reshape(3, 1)),
        "std3": np.full((3, 1), pcl_std[b], dtype=f32),
        "noiseT": np.ascontiguousarray(noi.transpose(0, 2, 1).reshape(6, M)),
        "dispT": np.ascontiguousarray(
            disp.transpose(2, 0, 1).reshape(3, NMOD * N)),
        "noisy_nat": np.ascontiguousarray(
            noisy.reshape(NT, 128, 3).transpose(1, 0, 2).reshape(128, NT * 3)),
        "seed_nat96": np.ascontiguousarray(np.tile(seed, (128, NT))),
        "neghalf3": np.full((3, 1), -0.5, dtype=f32),
        "disp_nat": np.ascontiguousarray(
            disp.reshape(NMOD, NT, 128, 3).transpose(2, 0, 1, 3)
            .reshape(128, NMOD * NT * 3)),
    }


_LAST_EXEC_NS = None


def kernel(pcl_noisy, pcl_clean, pcl_seeds, pcl_std, pred_disp, noise,
           trace=False):
    global _LAST_EXEC_NS
    nc = _get_nc(split_waits=True)
    in_maps = [
        _shard(b, pcl_noisy, pcl_clean, pcl_seeds, pcl_std, pred_disp, noise)
        for b in range(B)
    ]
    res = run_bass_kernel_spmd(nc, in_maps, core_ids=list(range(B)), trace=trace)
    _LAST_EXEC_NS = res.exec_time_ns
    per_mod = np.zeros(4, dtype=np.float64)
    for b in range(B):
        per_mod += res.results[b]["loss4"][:, 0].astype(np.float64)
    loss = np.float32((per_mod / B).sum())
    return (loss, loss)
